# revision 26
# baseline (speedup 1.0000x reference)
"""Masked dot-product attention (B=32, L=1024, D=64) on 8 Trainium2 NeuronCores.

Sharding: data-parallel over the batch dim, 4 batches per core.

Per batch, on device (fp32r = tf32-class matmuls, fp32 elsewhere):
  S    = Q K^T + mask     matmul pairs; valid_lens mask and the q-side ones
                          vector are baked host-side into a 65th contraction
                          row (contraction zero-padded to 128: K<=65 streams
                          at half rate on the PE).
  E    = exp(S/8)         ScalarE, PSUM->SBUF, scale folded in
  S^T  = K Q^T + mask^T   same operand tiles with lhsT/rhs swapped
  E^T  = exp(S^T/8)       fp32r output, feeds the O matmuls directly
  O'^T = [V|1]^T E^T      accumulating matmuls -> PSUM [65, 512] halves;
                          row 64 = softmax row-sums (ones column trick)
  O^T -> PE-transpose ->  [128, 65] tiles: col 64 = row sums in per-partition
                          layout; reciprocal + normalize
  W    = E * recip        VectorE tensor_scalar; per-tile DMA (attn_w)
  O    = O' * recip       VectorE; one DMA per batch

The q rows are processed in a permuted order (q = 8*p + r for tile r,
partition p); attn_w is written in final layout on device via rearranged
APs; attn_score is written tile-major and un-permuted on the host (pure
layout gather). Emission is software-pipelined so the PE always has
independent matmuls to issue while ScalarE works through the exps.

Host side only reshapes/transposes/shards numpy data; all output numerics
run on the NeuronCores.
"""
import sys
import numpy as np
from contextlib import ExitStack

if '/opt/trn_rl_repo' not in sys.path:
    sys.path.insert(0, '/opt/trn_rl_repo')

import concourse.bass as bass
import concourse.mybir as mybir
import concourse.tile as tile
from concourse import bacc
from concourse.bass_utils import run_bass_kernel_spmd
from concourse.masks import make_identity

FP32 = mybir.dt.float32
F32R = mybir.dt.float32r

B, L, D = 32, 1024, 64
NCORES = 8
BPC = B // NCORES          # batches per core
NT = L // 128              # 128-row tiles per batch
MASK_VALUE = -1000000.0

_cache = {}


def _emit(tc, qt_d, kt_d, v_d, w_d, o_d):
    nc = tc.nc
    Exp = mybir.ActivationFunctionType.Exp

    with ExitStack() as ctx:
        const = ctx.enter_context(tc.tile_pool(name="const", bufs=1))
        ident = const.tile([128, 128], FP32, name="ident")

        qk_pool = ctx.enter_context(tc.tile_pool(name="qk", bufs=3))
        v_pool = ctx.enter_context(tc.tile_pool(name="vp", bufs=3))
        e_pool = ctx.enter_context(tc.tile_pool(name="ep", bufs=2 * NT + 2))
        et_pool = ctx.enter_context(tc.tile_pool(name="etp", bufs=NT + 4))
        w_pool = ctx.enter_context(tc.tile_pool(name="wp", bufs=6))
        ot_pool = ctx.enter_context(tc.tile_pool(name="otp", bufs=2))
        acc_pool = ctx.enter_context(tc.tile_pool(name="accp", bufs=4 * NT))
        osb_pool = ctx.enter_context(tc.tile_pool(name="osbp", bufs=2))
        # PSUM (8 banks): sA [128,1024] x2 = 4, ob [65,512]/[128,65] x4 = 4
        sA_ps = ctx.enter_context(tc.tile_pool(name="sAps", bufs=2, space="PSUM"))
        ob_ps = ctx.enter_context(tc.tile_pool(name="obps", bufs=4, space="PSUM"))

        state = {}

        def load(b, split=False):
            if b >= BPC:
                return
            qt_sb = qk_pool.tile([128, L], F32R, name=f"qt{b}", tag="qt")
            kt_sb = qk_pool.tile([128, L], F32R, name=f"kt{b}", tag="kt")
            if split:
                # prologue: parallel queues + halves so the first S matmuls
                # unblock after ~1/4 of the input bytes
                nc.sync.dma_start(qt_sb[:, 0:512], qt_d[b][:, 0:512])
                nc.gpsimd.dma_start(kt_sb[:, 0:512], kt_d[b][:, 0:512])
                nc.sync.dma_start(qt_sb[:, 512:1024], qt_d[b][:, 512:1024])
                nc.gpsimd.dma_start(kt_sb[:, 512:1024], kt_d[b][:, 512:1024])
            else:
                nc.gpsimd.dma_start(qt_sb[:], qt_d[b])
                nc.gpsimd.dma_start(kt_sb[:], kt_d[b])
            v_sb = v_pool.tile([128, NT * 65], F32R, name=f"v{b}", tag="v")
            nc.gpsimd.dma_start(v_sb[:], v_d[b])
            state[b] = dict(qt=qt_sb, kt=kt_sb, v=v_sb,
                            e=[None] * NT, et=[None] * NT)

        def s_tile(b, q):
            if b >= BPC:
                return
            st = state[b]
            sp = sA_ps.tile([128, L], FP32, name=f"s{b}_{q}", tag="s")
            lhs = st["qt"][:, q * 128:(q + 1) * 128]
            nc.tensor.matmul(sp[:, 0:512], lhs, st["kt"][:, 0:512],
                             start=True, stop=True)
            nc.tensor.matmul(sp[:, 512:1024], lhs, st["kt"][:, 512:1024],
                             start=True, stop=True)
            e = e_pool.tile([128, L], FP32, name=f"e{b}_{q}", tag="e")
            if b >= BPC - 3:
                # late batches: row sums via the activation accumulator so the
                # attn_w tiles can be normalized and DMA'd out immediately,
                # instead of waiting for the O pipeline (kills the DMA tail)
                acc = acc_pool.tile([128, 1], FP32, name=f"a{b}_{q}", tag="a")
                nc.scalar.activation(e[:], sp[:], Exp, scale=0.125,
                                     accum_out=acc[:])
                rec = acc_pool.tile([128, 1], FP32, name=f"r{b}_{q}", tag="r")
                nc.vector.reciprocal(rec[:], acc[:])
                st.setdefault("rec", {})[q] = rec
                w = w_pool.tile([128, L], FP32, name=f"w{b}_{q}", tag="w")
                nc.vector.tensor_scalar_mul(w[:], e[:], rec[:])
                nc.sync.dma_start(
                    w_d[b].rearrange("(p r) k -> p r k", r=NT)[:, q, :], w[:])
            else:
                nc.scalar.activation(e[:], sp[:], Exp, scale=0.125)
            st["e"][q] = e

        def st_tile(b, k):
            if b >= BPC:
                return
            st = state[b]
            sp = sA_ps.tile([128, L], FP32, name=f"t{b}_{k}", tag="s")
            lhs = st["kt"][:, k * 128:(k + 1) * 128]
            nc.tensor.matmul(sp[:, 0:512], lhs, st["qt"][:, 0:512],
                             start=True, stop=True)
            nc.tensor.matmul(sp[:, 512:1024], lhs, st["qt"][:, 512:1024],
                             start=True, stop=True)
            et = et_pool.tile([128, L], F32R, name=f"et{b}_{k}", tag="et")
            nc.scalar.activation(et[:], sp[:], Exp, scale=0.125)
            st["et"][k] = et

        def o_mm(b, h, k):
            st = state[b]
            if k == 0:
                st.setdefault("op", {})[h] = ob_ps.tile(
                    [65, 512], FP32, name=f"ot{b}_{h}", tag="ob")
            op = st["op"][h]
            lhs = st["v"][:, k * 65:(k + 1) * 65]
            nc.tensor.matmul(op[:], lhs,
                             st["et"][k][:, h * 512:(h + 1) * 512],
                             start=(k == 0), stop=(k == NT - 1))
            if k == NT - 1:
                if "ot" not in st:
                    st["ot"] = ot_pool.tile([65, L], FP32, name=f"otsb{b}",
                                            tag="otsb")
                nc.vector.tensor_copy(
                    st["ot"][:, h * 512:(h + 1) * 512], op[:])

        def bk_tile(b, q):
            st = state[b]
            if "osb" not in st:
                st["osb"] = osb_pool.tile([128, NT * D], FP32,
                                          name=f"osb{b}", tag="o")
            bk = ob_ps.tile([128, 65], FP32, name=f"bk{b}_{q}", tag="ob")
            nc.tensor.transpose(bk[:], st["ot"][:, q * 128:(q + 1) * 128],
                                ident[0:65, 0:65])
            if b >= BPC - 3:
                rec = st["rec"][q]
            else:
                rec = acc_pool.tile([128, 1], FP32, name=f"r{b}_{q}", tag="r")
                nc.vector.reciprocal(rec[:], bk[:, 64:65])
            nc.vector.tensor_scalar_mul(st["osb"][:, q * D:(q + 1) * D],
                                        bk[:, 0:D], rec[:])
            if b < BPC - 3:
                w = w_pool.tile([128, L], FP32, name=f"w{b}_{q}", tag="w")
                nc.vector.tensor_scalar_mul(w[:], st["e"][q][:], rec[:])
                # tile q holds DRAM rows {NT*p + q}: stride-NT row scatter
                nc.sync.dma_start(
                    w_d[b].rearrange("(p r) k -> p r k", r=NT)[:, q, :], w[:])

        def o_dma(b):
            st = state[b]
            nc.sync.dma_start(
                o_d[b].rearrange("r p d -> p r d"),
                st["osb"][:].rearrange("p (r d) -> p r d", r=NT))

        # --- schedule: software-pipelined, PE kept dense ---
        load(0, split=True)
        load(1)
        make_identity(nc, ident[:])
        for j in range(NT):
            s_tile(0, j)
        for j in range(NT):
            st_tile(0, j)
        for b in range(BPC):
            load(b + 2)
            # O^T half 0 woven with next batch's S tiles
            for k in range(NT):
                o_mm(b, 0, k)
                if k % 2 == 1:
                    s_tile(b + 1, k // 2)
            # O^T half 1 woven with first BK tiles and remaining S tiles
            for k in range(NT):
                o_mm(b, 1, k)
                if k % 2 == 0:
                    bk_tile(b, k // 2)
                else:
                    s_tile(b + 1, 4 + k // 2)
            # tail: remaining BK tiles woven with next batch's S^T tiles
            for j in range(NT):
                if j < 4:
                    bk_tile(b, 4 + j)
                st_tile(b + 1, j)
            o_dma(b)


def _build():
    if "nc" in _cache:
        return _cache["nc"]
    nc = bacc.Bacc("TRN2", debug=False, num_devices=NCORES)
    qt_d = nc.dram_tensor("qt_in", [BPC, 128, L], F32R, kind="ExternalInput").ap()
    kt_d = nc.dram_tensor("kt_in", [BPC, 128, L], F32R, kind="ExternalInput").ap()
    v_d = nc.dram_tensor("v_in", [BPC, 128, NT * 65], F32R,
                         kind="ExternalInput").ap()
    w_d = nc.dram_tensor("w_out", [BPC, L, L], FP32, kind="ExternalOutput").ap()
    o_d = nc.dram_tensor("o_out", [BPC, NT, 128, D], FP32,
                         kind="ExternalOutput").ap()
    with tile.TileContext(nc) as tc:
        _emit(tc, qt_d, kt_d, v_d, w_d, o_d)
    nc.compile()
    _cache["nc"] = nc
    return nc


def _prep(queries, keys, values, valid_lens):
    q = np.asarray(queries, dtype=np.float32)
    k = np.asarray(keys, dtype=np.float32)
    v = np.asarray(values, dtype=np.float32)
    vl = np.asarray(valid_lens, dtype=np.int32)

    # [B, 65, L]: rows 0..63 = X^T; row 64 = ones (q side) / mask row
    # (k side). On device the SBUF tile is zero-padded to K=128 (full-rate
    # contraction streaming where K<=65 runs at half rate).
    # q columns permuted so column c = r*128 + p maps to q = 8*p + r.
    qt = np.zeros((B, 128, L), dtype=np.float32)
    qt[:, :64, :] = (q.transpose(0, 2, 1)
                     .reshape(B, 64, 128, NT).transpose(0, 1, 3, 2)
                     .reshape(B, 64, L))
    qt[:, 64, :] = 1.0
    kt = np.zeros((B, 128, L), dtype=np.float32)
    kt[:, :64, :] = k.transpose(0, 2, 1)
    kt[:, 64, :] = np.where(np.arange(L)[None, :] < vl[:, None], 0.0,
                            MASK_VALUE)
    # [V | 1] — ones column yields softmax row-sums; pre-swizzled to the
    # SBUF layout [128, k*65] so the load is one contiguous run per partition
    vp = np.empty((B, L, 65), dtype=np.float32)
    vp[:, :, :64] = v
    vp[:, :, 64] = 1.0
    vp = (vp.reshape(B, NT, 128, 65).transpose(0, 2, 1, 3)
          .reshape(B, 128, NT * 65))
    return qt, kt, vp


def kernel(queries, keys, values, valid_lens, _want_time=False):
    nc = _build()
    qt, kt, vp = _prep(queries, keys, values, valid_lens)
    in_maps = []
    for c in range(NCORES):
        s = slice(c * BPC, (c + 1) * BPC)
        in_maps.append({
            "qt_in": np.ascontiguousarray(qt[s]),
            "kt_in": np.ascontiguousarray(kt[s]),
            "v_in": np.ascontiguousarray(vp[s]),
        })
    res = run_bass_kernel_spmd(nc, in_maps, list(range(NCORES)),
                               trace=_want_time)
    attn_score = np.empty((B, L, D), dtype=np.float32)
    attn_w = np.empty((B, L, L), dtype=np.float32)
    for c in range(NCORES):
        s = slice(c * BPC, (c + 1) * BPC)
        attn_w[s] = res.results[c]["w_out"]
        # o_out is [BPC, r, p, D] with q = 8*p + r
        o = res.results[c]["o_out"]
        attn_score[s] = o.transpose(0, 2, 1, 3).reshape(BPC, L, D)
    if _want_time:
        return (attn_score, attn_w), res
    return (attn_score, attn_w)


# revision 27
# speedup vs baseline: 1.0585x; 1.0585x over previous
"""Masked dot-product attention (B=32, L=1024, D=64) on 8 Trainium2 NeuronCores.

Sharding: data-parallel over the batch dim, 4 batches per core.

Per batch, on device (fp32r = tf32-class matmuls, fp32 elsewhere):
  S    = Q K^T + mask     matmul pairs; valid_lens mask and the q-side ones
                          vector are baked host-side into a 65th contraction
                          row (contraction zero-padded to 128: K<=65 streams
                          at half rate on the PE).
  E    = exp(S/8)         ScalarE, PSUM->SBUF, scale folded in
  S^T  = K Q^T + mask^T   same operand tiles with lhsT/rhs swapped
  E^T  = exp(S^T/8)       fp32r output, feeds the O matmuls directly
  O'^T = [V|1]^T E^T      accumulating matmuls -> PSUM [65, 512] halves;
                          row 64 = softmax row-sums (ones column trick)
  O^T -> PE-transpose ->  [128, 65] tiles: col 64 = row sums in per-partition
                          layout; reciprocal + normalize
  W    = E * recip        VectorE tensor_scalar; per-tile DMA (attn_w)
  O    = O' * recip       VectorE; one DMA per batch

The q rows are processed in a permuted order (q = 8*p + r for tile r,
partition p); attn_w is written in final layout on device via rearranged
APs; attn_score is written tile-major and un-permuted on the host (pure
layout gather). Emission is software-pipelined so the PE always has
independent matmuls to issue while ScalarE works through the exps.

Host side only reshapes/transposes/shards numpy data; all output numerics
run on the NeuronCores.
"""
import sys
import numpy as np
from contextlib import ExitStack

if '/opt/trn_rl_repo' not in sys.path:
    sys.path.insert(0, '/opt/trn_rl_repo')

import concourse.bass as bass
import concourse.mybir as mybir
import concourse.tile as tile
from concourse import bacc
from concourse.bass_utils import run_bass_kernel_spmd
from concourse.masks import make_identity

FP32 = mybir.dt.float32
F32R = mybir.dt.float32r

B, L, D = 32, 1024, 64
NCORES = 8
BPC = B // NCORES          # batches per core
NT = L // 128              # 128-row tiles per batch
MASK_VALUE = -1000000.0

_cache = {}


def _emit(tc, qt_d, kt_d, v_d, w_d, o_d):
    nc = tc.nc
    Exp = mybir.ActivationFunctionType.Exp

    with ExitStack() as ctx:
        const = ctx.enter_context(tc.tile_pool(name="const", bufs=1))
        ident = const.tile([128, 128], FP32, name="ident")

        qk_pool = ctx.enter_context(tc.tile_pool(name="qk", bufs=3))
        v_pool = ctx.enter_context(tc.tile_pool(name="vp", bufs=3))
        e_pool = ctx.enter_context(tc.tile_pool(name="ep", bufs=2 * NT + 2))
        et_pool = ctx.enter_context(tc.tile_pool(name="etp", bufs=NT + 4))
        w_pool = ctx.enter_context(tc.tile_pool(name="wp", bufs=6))
        ot_pool = ctx.enter_context(tc.tile_pool(name="otp", bufs=2))
        acc_pool = ctx.enter_context(tc.tile_pool(name="accp", bufs=4 * NT))
        osb_pool = ctx.enter_context(tc.tile_pool(name="osbp", bufs=2))
        # PSUM (8 banks): sA [128,1024] x2 = 4, ob [65,512]/[128,65] x4 = 4
        sA_ps = ctx.enter_context(tc.tile_pool(name="sAps", bufs=3, space="PSUM"))
        ob_ps = ctx.enter_context(tc.tile_pool(name="obps", bufs=2, space="PSUM"))

        state = {}

        def load(b, split=False):
            if b >= BPC:
                return
            qt_sb = qk_pool.tile([128, L], F32R, name=f"qt{b}", tag="qt")
            kt_sb = qk_pool.tile([128, L], F32R, name=f"kt{b}", tag="kt")
            if split:
                # prologue: parallel queues + halves so the first S matmuls
                # unblock after ~1/4 of the input bytes
                nc.sync.dma_start(qt_sb[:, 0:512], qt_d[b][:, 0:512])
                nc.gpsimd.dma_start(kt_sb[:, 0:512], kt_d[b][:, 0:512])
                nc.sync.dma_start(qt_sb[:, 512:1024], qt_d[b][:, 512:1024])
                nc.gpsimd.dma_start(kt_sb[:, 512:1024], kt_d[b][:, 512:1024])
            else:
                nc.gpsimd.dma_start(qt_sb[:], qt_d[b])
                nc.gpsimd.dma_start(kt_sb[:], kt_d[b])
            v_sb = v_pool.tile([128, NT * 65], F32R, name=f"v{b}", tag="v")
            nc.gpsimd.dma_start(v_sb[:], v_d[b])
            state[b] = dict(qt=qt_sb, kt=kt_sb, v=v_sb,
                            e=[None] * NT, et=[None] * NT)

        def s_tile(b, q):
            if b >= BPC:
                return
            st = state[b]
            sp = sA_ps.tile([128, L], FP32, name=f"s{b}_{q}", tag="s")
            lhs = st["qt"][:, q * 128:(q + 1) * 128]
            nc.tensor.matmul(sp[:, 0:512], lhs, st["kt"][:, 0:512],
                             start=True, stop=True)
            nc.tensor.matmul(sp[:, 512:1024], lhs, st["kt"][:, 512:1024],
                             start=True, stop=True)
            e = e_pool.tile([128, L], FP32, name=f"e{b}_{q}", tag="e")
            if b >= BPC - 2:
                # late batches: row sums via the activation accumulator so the
                # attn_w tiles can be normalized and DMA'd out immediately,
                # instead of waiting for the O pipeline (kills the DMA tail)
                acc = acc_pool.tile([128, 1], FP32, name=f"a{b}_{q}", tag="a")
                nc.scalar.activation(e[:], sp[:], Exp, scale=0.125,
                                     accum_out=acc[:])
                rec = acc_pool.tile([128, 1], FP32, name=f"r{b}_{q}", tag="r")
                nc.vector.reciprocal(rec[:], acc[:])
                st.setdefault("rec", {})[q] = rec
                w = w_pool.tile([128, L], FP32, name=f"w{b}_{q}", tag="w")
                nc.vector.tensor_scalar_mul(w[:], e[:], rec[:])
                nc.sync.dma_start(
                    w_d[b].rearrange("(p r) k -> p r k", r=NT)[:, q, :], w[:])
            else:
                nc.scalar.activation(e[:], sp[:], Exp, scale=0.125)
            st["e"][q] = e

        def st_tile(b, k):
            if b >= BPC:
                return
            st = state[b]
            sp = sA_ps.tile([128, L], FP32, name=f"t{b}_{k}", tag="s")
            lhs = st["kt"][:, k * 128:(k + 1) * 128]
            nc.tensor.matmul(sp[:, 0:512], lhs, st["qt"][:, 0:512],
                             start=True, stop=True)
            nc.tensor.matmul(sp[:, 512:1024], lhs, st["qt"][:, 512:1024],
                             start=True, stop=True)
            et = et_pool.tile([128, L], F32R, name=f"et{b}_{k}", tag="et")
            nc.scalar.activation(et[:], sp[:], Exp, scale=0.125)
            st["et"][k] = et

        def o_mm(b, h, k):
            st = state[b]
            if k == 0:
                st.setdefault("op", {})[h] = ob_ps.tile(
                    [65, 512], FP32, name=f"ot{b}_{h}", tag="ob")
            op = st["op"][h]
            lhs = st["v"][:, k * 65:(k + 1) * 65]
            nc.tensor.matmul(op[:], lhs,
                             st["et"][k][:, h * 512:(h + 1) * 512],
                             start=(k == 0), stop=(k == NT - 1))
            if k == NT - 1:
                if "ot" not in st:
                    st["ot"] = ot_pool.tile([65, L], FP32, name=f"otsb{b}",
                                            tag="otsb")
                nc.vector.tensor_copy(
                    st["ot"][:, h * 512:(h + 1) * 512], op[:])

        def bk_tile(b, q):
            st = state[b]
            if "osb" not in st:
                st["osb"] = osb_pool.tile([128, NT * D], FP32,
                                          name=f"osb{b}", tag="o")
            bk = ob_ps.tile([128, 65], FP32, name=f"bk{b}_{q}", tag="ob")
            nc.tensor.transpose(bk[:], st["ot"][:, q * 128:(q + 1) * 128],
                                ident[0:65, 0:65])
            if b >= BPC - 2:
                rec = st["rec"][q]
            else:
                rec = acc_pool.tile([128, 1], FP32, name=f"r{b}_{q}", tag="r")
                nc.vector.reciprocal(rec[:], bk[:, 64:65])
            nc.vector.tensor_scalar_mul(st["osb"][:, q * D:(q + 1) * D],
                                        bk[:, 0:D], rec[:])
            if b < BPC - 2:
                w = w_pool.tile([128, L], FP32, name=f"w{b}_{q}", tag="w")
                nc.vector.tensor_scalar_mul(w[:], st["e"][q][:], rec[:])
                # tile q holds DRAM rows {NT*p + q}: stride-NT row scatter
                nc.sync.dma_start(
                    w_d[b].rearrange("(p r) k -> p r k", r=NT)[:, q, :], w[:])

        def o_dma(b):
            st = state[b]
            nc.sync.dma_start(
                o_d[b].rearrange("r p d -> p r d"),
                st["osb"][:].rearrange("p (r d) -> p r d", r=NT))

        # --- schedule: software-pipelined, PE kept dense ---
        load(0, split=True)
        load(1)
        make_identity(nc, ident[:])
        for j in range(NT):
            s_tile(0, j)
        for j in range(NT):
            st_tile(0, j)
        for b in range(BPC):
            load(b + 2)
            # O^T half 0 woven with next batch's S tiles
            for k in range(NT):
                o_mm(b, 0, k)
                if k % 2 == 1:
                    s_tile(b + 1, k // 2)
            # O^T half 1 woven with first BK tiles and remaining S tiles
            for k in range(NT):
                o_mm(b, 1, k)
                if k % 2 == 0:
                    bk_tile(b, k // 2)
                else:
                    s_tile(b + 1, 4 + k // 2)
            # tail: remaining BK tiles woven with next batch's S^T tiles
            for j in range(NT):
                if j < 4:
                    bk_tile(b, 4 + j)
                st_tile(b + 1, j)
            o_dma(b)


def _build():
    if "nc" in _cache:
        return _cache["nc"]
    nc = bacc.Bacc("TRN2", debug=False, num_devices=NCORES)
    qt_d = nc.dram_tensor("qt_in", [BPC, 128, L], F32R, kind="ExternalInput").ap()
    kt_d = nc.dram_tensor("kt_in", [BPC, 128, L], F32R, kind="ExternalInput").ap()
    v_d = nc.dram_tensor("v_in", [BPC, 128, NT * 65], F32R,
                         kind="ExternalInput").ap()
    w_d = nc.dram_tensor("w_out", [BPC, L, L], FP32, kind="ExternalOutput").ap()
    o_d = nc.dram_tensor("o_out", [BPC, NT, 128, D], FP32,
                         kind="ExternalOutput").ap()
    with tile.TileContext(nc) as tc:
        _emit(tc, qt_d, kt_d, v_d, w_d, o_d)
    nc.compile()
    _cache["nc"] = nc
    return nc


def _prep(queries, keys, values, valid_lens):
    q = np.asarray(queries, dtype=np.float32)
    k = np.asarray(keys, dtype=np.float32)
    v = np.asarray(values, dtype=np.float32)
    vl = np.asarray(valid_lens, dtype=np.int32)

    # [B, 65, L]: rows 0..63 = X^T; row 64 = ones (q side) / mask row
    # (k side). On device the SBUF tile is zero-padded to K=128 (full-rate
    # contraction streaming where K<=65 runs at half rate).
    # q columns permuted so column c = r*128 + p maps to q = 8*p + r.
    qt = np.zeros((B, 128, L), dtype=np.float32)
    qt[:, :64, :] = (q.transpose(0, 2, 1)
                     .reshape(B, 64, 128, NT).transpose(0, 1, 3, 2)
                     .reshape(B, 64, L))
    qt[:, 64, :] = 1.0
    kt = np.zeros((B, 128, L), dtype=np.float32)
    kt[:, :64, :] = k.transpose(0, 2, 1)
    kt[:, 64, :] = np.where(np.arange(L)[None, :] < vl[:, None], 0.0,
                            MASK_VALUE)
    # [V | 1] — ones column yields softmax row-sums; pre-swizzled to the
    # SBUF layout [128, k*65] so the load is one contiguous run per partition
    vp = np.empty((B, L, 65), dtype=np.float32)
    vp[:, :, :64] = v
    vp[:, :, 64] = 1.0
    vp = (vp.reshape(B, NT, 128, 65).transpose(0, 2, 1, 3)
          .reshape(B, 128, NT * 65))
    return qt, kt, vp


def kernel(queries, keys, values, valid_lens, _want_time=False):
    nc = _build()
    qt, kt, vp = _prep(queries, keys, values, valid_lens)
    in_maps = []
    for c in range(NCORES):
        s = slice(c * BPC, (c + 1) * BPC)
        in_maps.append({
            "qt_in": np.ascontiguousarray(qt[s]),
            "kt_in": np.ascontiguousarray(kt[s]),
            "v_in": np.ascontiguousarray(vp[s]),
        })
    res = run_bass_kernel_spmd(nc, in_maps, list(range(NCORES)),
                               trace=_want_time)
    attn_score = np.empty((B, L, D), dtype=np.float32)
    attn_w = np.empty((B, L, L), dtype=np.float32)
    for c in range(NCORES):
        s = slice(c * BPC, (c + 1) * BPC)
        attn_w[s] = res.results[c]["w_out"]
        # o_out is [BPC, r, p, D] with q = 8*p + r
        o = res.results[c]["o_out"]
        attn_score[s] = o.transpose(0, 2, 1, 3).reshape(BPC, L, D)
    if _want_time:
        return (attn_score, attn_w), res
    return (attn_score, attn_w)
